# revision 1
# baseline (speedup 1.0000x reference)
"""ConvTransE forward on 8 Trainium2 NeuronCores (Bass/Tile) — bf16 PE, v5.

Math shortcut: the reference computes scores = x @ ent.T ([B, 100000]) and
returns scores[i, t[i]]; we only compute out[b] = x[b] . ent[t[b]].
The conv's retained slice [:, :512] depends only on ent[h] and rel[r][:, 0].

Sharding: tensor-parallel over the projection contraction dim (channels).
Core m owns channels [4m, 4m+4).  Every core:
  - gathers ent[h] rows (cast fp32->bf16 in the DMA), rel[r][:, 0] only,
    and ent[t] rows (fp32) via indirect DMA,
  - PE-transposes overlapping 128-wide windows of the gathered rows (bf16),
  - runs the conv as banded matmuls on the PE (bands built on host, bf16),
  - projects its K-slice:  z_m = relu(conv)_m @ proj_w_m^T  (K = 2048/core),
  - emits partial[b] = z_m[b] . ent[t[b]] via a fused multiply+row-sum (fp32).
proj_b rides along as an extra "ones" contraction row on core 0 only.
Host sums the 8 [2048] partials.

v7: bf16 entity table in DRAM (halves gather HBM reads; t-rows
upcast to fp32 in the DMA), pwt tail loads gated behind the first
h-gathers via WAW dummies, chunk 0 interleaves the head of its own
first projection group into its conv phase.
v6: grouped per-btl projection (no per-matmul PSUM bank cycling — that
measured ~20% slower per matmul), conv PSUM tiles hold 2 segments and
relu+bias runs once per pair (ACT/DVE alternating), and each chunk's
4th projection group is deferred into the middle of the NEXT chunk's
conv phase so the conv relu-drain latency is hidden behind projection
matmuls instead of stalling the in-order PE queue. Gathers prefetched
one chunk ahead (h, then rel, then t), index DMAs before weight DMAs,
pwt in two DMAs, stub transposes packed into each btl's transpose tile.
"""

import numpy as np

NE, NRR, D, C, B = 100000, 500, 512, 32, 2048
NCORES = 8
CPC = C // NCORES          # 4 channels per core
NQ = B // 128              # 16 batch tiles of 128
CHUNK_BT = 4               # batch tiles per pipeline chunk
NCHUNK = NQ // CHUNK_BT
NB = CHUNK_BT * 128        # 512 batch columns per chunk
JB = 126                   # conv j-block (126 outputs need a 128-wide input window)
NSEG = 16                  # (c, s) main contraction blocks per core
KSTUB = CPC * 8 + 1        # 33: packed j=504..511 stub rows + ones row
GWW = 640                  # per-btl transpose tile: 4 windows * 128 + stub 128

_CACHE = {}


def _build_nc():
    from contextlib import ExitStack

    import concourse.bass as bass
    import concourse.tile as tile
    from concourse import bacc, mybir
    from concourse.masks import make_identity

    f32 = mybir.dt.float32
    bf16 = mybir.dt.bfloat16
    i32 = mybir.dt.int32
    Alu = mybir.AluOpType

    nc = bacc.Bacc("TRN2", target_bir_lowering=False, debug=False,
                   num_devices=NCORES)

    ent = nc.dram_tensor("ent", [NE, D], bf16, kind="ExternalInput")
    rel = nc.dram_tensor("rel", [NRR, D], f32, kind="ExternalInput")
    hI = nc.dram_tensor("hI", [128, NQ], i32, kind="ExternalInput")
    tI = nc.dram_tensor("tI", [128, NQ], i32, kind="ExternalInput")
    rI = nc.dram_tensor("rI", [128, NQ], i32, kind="ExternalInput")
    band = nc.dram_tensor("band", [128, NSEG * JB], bf16, kind="ExternalInput")
    bstub = nc.dram_tensor("bstub", [10, 32], bf16, kind="ExternalInput")
    pwt = nc.dram_tensor("pwt", [JB, NSEG * D], bf16, kind="ExternalInput")
    pstub = nc.dram_tensor("pstub", [KSTUB, D], bf16, kind="ExternalInput")
    cbias = nc.dram_tensor("cbias", [128, CPC], f32, kind="ExternalInput")
    sbias = nc.dram_tensor("sbias", [32, 1], f32, kind="ExternalInput")
    out = nc.dram_tensor("out", [128, NQ], f32, kind="ExternalOutput")

    with tile.TileContext(nc) as tc, ExitStack() as ctx:
        const = ctx.enter_context(tc.tile_pool(name="const", bufs=1))
        gpad_p = ctx.enter_context(tc.tile_pool(name="gpad", bufs=10))
        v_p = ctx.enter_context(tc.tile_pool(name="vt", bufs=12))
        gw_p = ctx.enter_context(tc.tile_pool(name="gw", bufs=2))
        gws_p = ctx.enter_context(tc.tile_pool(name="gws", bufs=2))
        y_p = ctx.enter_context(tc.tile_pool(name="ym", bufs=2))
        ys_p = ctx.enter_context(tc.tile_pool(name="ys", bufs=2))
        sc_p = ctx.enter_context(tc.tile_pool(name="scr", bufs=2))
        tp_p = ctx.enter_context(tc.tile_pool(name="tp", bufs=2, space="PSUM"))
        yp_p = ctx.enter_context(tc.tile_pool(name="yp", bufs=2, space="PSUM"))
        z_p = ctx.enter_context(tc.tile_pool(name="zp", bufs=2, space="PSUM"))

        # tiny index tables first: the first gathers depend only on these
        hI_sb = const.tile([128, NQ], i32)
        nc.sync.dma_start(hI_sb[:], hI[:])
        rI_sb = const.tile([128, NQ], i32)
        nc.sync.dma_start(rI_sb[:], rI[:])
        ident = const.tile([128, 128], bf16)
        make_identity(nc, ident[:])
        band_sb = const.tile([128, NSEG * JB], bf16)
        nc.sync.dma_start(band_sb[:], band[:])
        pwt_sb = const.tile([JB, NSEG * D], bf16)
        nc.sync.dma_start(pwt_sb[:, 0:4 * D], pwt[:, 0:4 * D])
        bstub_sb = const.tile([10, 32], bf16)
        nc.sync.dma_start(bstub_sb[:], bstub[:])
        cb_sb = const.tile([128, CPC], f32)
        nc.sync.dma_start(cb_sb[:], cbias[:])
        sb_sb = const.tile([32, 1], f32)
        nc.sync.dma_start(sb_sb[:], sbias[:])
        tI_sb = const.tile([128, NQ], i32)
        nc.sync.dma_start(tI_sb[:], tI[:])
        pstub_sb = const.tile([KSTUB, D], bf16)
        nc.sync.dma_start(pstub_sb[:], pstub[:])
        out_sb = const.tile([128, NQ], f32)

        def emit_gathers(c):
            gpads, vts = [], []
            for btl in range(CHUNK_BT):
                q = c * CHUNK_BT + btl
                gpad = gpad_p.tile([128, D + 2], bf16, name="gpad")
                nc.vector.memset(gpad[:, 0:1], 0.0)
                nc.gpsimd.indirect_dma_start(
                    out=gpad[:, 1:D + 1], out_offset=None, in_=ent[:],
                    in_offset=bass.IndirectOffsetOnAxis(
                        ap=hI_sb[:, q:q + 1], axis=0))
                gpads.append(gpad)
            for btl in range(CHUNK_BT):
                q = c * CHUNK_BT + btl
                nc.gpsimd.indirect_dma_start(
                    out=gpads[btl][:, D + 1:D + 2], out_offset=None,
                    in_=rel[:, 0:1],
                    in_offset=bass.IndirectOffsetOnAxis(
                        ap=rI_sb[:, q:q + 1], axis=0))
            for btl in range(CHUNK_BT):
                q = c * CHUNK_BT + btl
                vt = v_p.tile([128, D], f32, name="vt")
                nc.gpsimd.indirect_dma_start(
                    out=vt[:], out_offset=None, in_=ent[:],
                    in_offset=bass.IndirectOffsetOnAxis(
                        ap=tI_sb[:, q:q + 1], axis=0))
                vts.append(vt)
            return gpads, vts

        def emit_proj_head(q, ym, hi):
            z = z_p.tile([128, D], f32, name="zt")
            b = q % CHUNK_BT
            for i in range(hi):
                nc.tensor.matmul(
                    z[:], ym[:, i * NB + b * 128:i * NB + (b + 1) * 128],
                    pwt_sb[:, i * D:(i + 1) * D],
                    start=(i == 0), stop=False)
            return z

        def emit_proj(q, ym, ystub, vt, z=None, lo=0):
            b = q % CHUNK_BT
            if z is None:
                z = z_p.tile([128, D], f32, name="zt")
            for i in range(lo, NSEG):
                nc.tensor.matmul(
                    z[:], ym[:, i * NB + b * 128:i * NB + (b + 1) * 128],
                    pwt_sb[:, i * D:(i + 1) * D],
                    start=(i == 0), stop=False)
            nc.tensor.matmul(z[:], ystub[:, b * 128:(b + 1) * 128],
                             pstub_sb[:], start=False, stop=True)
            scr = sc_p.tile([128, D], f32)
            nc.vector.scalar_tensor_tensor(
                out=scr[:], in0=z[:], scalar=1.0, in1=vt[:],
                op0=Alu.mult, op1=Alu.mult,
                accum_out=out_sb[:, q:q + 1])

        pending = emit_gathers(0)
        # gate the pwt tail loads on the first h-gather landing so the
        # chunk-0 gathers aren't starved of HBM by the 2MB weight stream
        nc.vector.tensor_copy(pwt_sb[0:1, 4 * D:4 * D + 1],
                              pending[0][0][0:1, 1:2])
        nc.sync.dma_start(pwt_sb[:, 4 * D:10 * D], pwt[:, 4 * D:10 * D])
        nc.vector.tensor_copy(pwt_sb[0:1, 10 * D:10 * D + 1],
                              pending[0][1][0:1, 1:2])
        nc.sync.dma_start(pwt_sb[:, 10 * D:], pwt[:, 10 * D:])
        deferred = None    # (q, ym, ystub, vt) for the previous chunk's b3
        for chunk in range(NCHUNK):
            gpads, vts = pending
            if chunk + 1 < NCHUNK:
                pending = emit_gathers(chunk + 1)

            # per-btl: 4 main window transposes + stub transpose into one
            # PSUM tile, one main copy into btl-major gw, one stub copy
            gw = gw_p.tile([128, CHUNK_BT * D], bf16)
            gwv = gw[:].rearrange("p (b s c) -> p b s c",
                                  b=CHUNK_BT, s=4, c=128)
            gws = gws_p.tile([10, NB], bf16)
            for btl in range(CHUNK_BT):
                gpad = gpads[btl]
                tp = tp_p.tile([128, GWW], bf16)
                for s in range(4):
                    nc.tensor.transpose(tp[:, s * 128:(s + 1) * 128],
                                        gpad[:, JB * s:JB * s + 128], ident[:])
                nc.tensor.transpose(tp[0:10, 512:640],
                                    gpad[:, 4 * JB:D + 2], ident[:])
                nc.vector.tensor_copy(gw[:, btl * D:(btl + 1) * D],
                                      tp[:, 0:512])
                nc.vector.tensor_copy(gws[:, btl * 128:(btl + 1) * 128],
                                      tp[0:10, 512:640])

            ym = y_p.tile([JB, NSEG * NB], bf16)
            ystub = ys_p.tile([KSTUB, NB], bf16)
            nc.vector.memset(ystub[32:33, :], 1.0)

            # conv: 2 segments per PSUM tile, one relu+bias op per pair
            def conv_pair(k):
                yp = yp_p.tile([JB, 2 * NB], f32, name="yp")
                for j in range(2):
                    cs = 2 * k + j
                    s4 = cs % 4
                    nc.tensor.matmul(yp[:, j * NB:(j + 1) * NB],
                                     band_sb[:, cs * JB:(cs + 1) * JB],
                                     gwv[:, :, s4, :], start=True, stop=True)
                c4 = (2 * k) // 4
                if k % 2 == 0:
                    nc.scalar.activation(
                        ym[:, 2 * k * NB:(2 * k + 2) * NB], yp[:],
                        mybir.ActivationFunctionType.Relu,
                        bias=cb_sb[0:JB, c4:c4 + 1])
                else:
                    nc.vector.tensor_scalar(ym[:, 2 * k * NB:(2 * k + 2) * NB],
                                            yp[:], cb_sb[0:JB, c4:c4 + 1],
                                            0.0, Alu.add, Alu.max)

            for k in range(4):
                conv_pair(k)
            # previous chunk's 4th projection group fills the PE while this
            # chunk's first relus drain; chunk 0 uses the head of its own
            # first group instead
            z0 = None
            if deferred is not None:
                emit_proj(*deferred)
            else:
                z0 = emit_proj_head(chunk * CHUNK_BT, ym, 8)
            for k in range(4, NSEG // 2):
                conv_pair(k)
            yps = yp_p.tile([JB, 2 * NB], f32, name="yp")
            nc.tensor.matmul(yps[0:32, 0:NB], bstub_sb[:], gws[:],
                             start=True, stop=True)
            nc.scalar.activation(ystub[0:32, :], yps[0:32, 0:NB],
                                 mybir.ActivationFunctionType.Relu,
                                 bias=sb_sb[:, 0:1])

            first = 0
            if z0 is not None:
                emit_proj(chunk * CHUNK_BT, ym, ystub, vts[0], z=z0, lo=8)
                first = 1
            for btl in range(first, CHUNK_BT - 1):
                emit_proj(chunk * CHUNK_BT + btl, ym, ystub, vts[btl])
            deferred = (chunk * CHUNK_BT + 3, ym, ystub, vts[3])

        emit_proj(*deferred)
        nc.sync.dma_start(out[:], out_sb[:])
    nc.finalize()
    return nc


def _host_prep(inputs):
    """Per-core input dicts from the full problem inputs."""
    import ml_dtypes

    bf = ml_dtypes.bfloat16
    ent = np.ascontiguousarray(
        np.asarray(inputs["ent"], dtype=np.float32).astype(bf))
    rel = np.ascontiguousarray(np.asarray(inputs["rel"], dtype=np.float32))
    w = np.asarray(inputs["conv_w"], dtype=np.float32)       # [32, 1, 3]
    cb = np.asarray(inputs["conv_b"], dtype=np.float32)      # [32]
    pw = np.asarray(inputs["proj_w"], dtype=np.float32)      # [512, 16384]
    pb = np.asarray(inputs["proj_b"], dtype=np.float32)      # [512]
    h = np.asarray(inputs["h"]).astype(np.int32)
    r = np.asarray(inputs["r"]).astype(np.int32)
    t = np.asarray(inputs["t"]).astype(np.int32)

    hI = np.ascontiguousarray(h.reshape(NQ, 128).T)
    rI = np.ascontiguousarray(r.reshape(NQ, 128).T)
    tI = np.ascontiguousarray(t.reshape(NQ, 128).T)

    jl = np.arange(JB)
    jl8 = np.arange(8)
    in_maps = []
    for m in range(NCORES):
        band = np.zeros((128, NSEG, JB), np.float32)
        bstub = np.zeros((10, 32), np.float32)
        pwt = np.zeros((JB, NSEG, D), np.float32)
        pstub = np.zeros((KSTUB, D), np.float32)
        cbias = np.zeros((128, CPC), np.float32)
        sbias = np.zeros((32, 1), np.float32)
        for c in range(CPC):
            cg = CPC * m + c
            cbias[:, c] = cb[cg]
            sbias[c * 8:(c + 1) * 8, 0] = cb[cg]
            for k in range(3):
                bstub[jl8 + k, c * 8 + jl8] = w[cg, 0, k]
            for s in range(4):
                cs = c * 4 + s
                for k in range(3):
                    band[jl + k, cs, jl] = w[cg, 0, k]
                pwt[:, cs, :] = pw[:, cg * D + JB * s: cg * D + JB * (s + 1)].T
            pstub[c * 8:(c + 1) * 8, :] = pw[:, cg * D + 504: cg * D + 512].T
        if m == 0:
            pstub[32] = pb
        in_maps.append({
            "ent": ent, "rel": rel, "hI": hI, "tI": tI, "rI": rI,
            "band": np.ascontiguousarray(band.reshape(128, NSEG * JB)).astype(bf),
            "bstub": bstub.astype(bf),
            "pwt": np.ascontiguousarray(pwt.reshape(JB, NSEG * D)).astype(bf),
            "pstub": pstub.astype(bf), "cbias": cbias, "sbias": sbias,
        })
    return in_maps


def _run(inputs, trace=False, tmpdir=None):
    from concourse.bass_utils import run_bass_kernel_spmd

    if "nc" not in _CACHE:
        _CACHE["nc"] = _build_nc()
    nc = _CACHE["nc"]
    in_maps = _host_prep(inputs)
    res = run_bass_kernel_spmd(nc, in_maps, core_ids=list(range(NCORES)),
                               trace=trace, tmpdir=tmpdir)
    total = np.zeros((128, NQ), np.float64)
    for mres in res.results:
        total += mres["out"].astype(np.float64)
    return total.T.reshape(B).astype(np.float32), res


def kernel(**inputs):
    out, _ = _run(inputs, trace=False)
    return out



# revision 2
# speedup vs baseline: 3.3714x; 3.3714x over previous
"""ConvTransE forward on 8 Trainium2 NeuronCores (Bass/Tile) — v8 "relu fold".

Math: the reference returns out[b] = z[b] . ent[t[b]] with
z = relu(conv(x) + cb) @ proj_w.T + pb, x = [ent[h], rel[r][:,0-only]].
Because |ent| <= 0.0077 while conv_b ~ U(-0.58, 0.58), for most channels
relu is provably the identity (cb >= max possible |conv|) or provably
zero (cb <= -max).  Identity channels fold EXACTLY into one linear map
W_fold [513+ones, 512] built on host from the weights; zero channels
drop; only the few genuinely nonlinear channels keep the
conv->relu->proj path on device.  With the staged weights that is 23
linear / 8 zero / 1 nonlinear channel.

Sharding: data-parallel over batch.  Core m owns batch tiles 2m, 2m+1
(256 rows).  Per tile: indirect-gather ent[h] rows (bf16), PE-transpose
126-stride windows, z = xT.T @ W_fold (+ stub for x[503..512], rel,
ones/const rows), plus per-NL-channel banded conv + relu + projection,
then out[b] = z[b] . ent[t[b]] via fused multiply+row-sum.  No
cross-core reduction; host concatenates the per-core [128, 2] outputs.
rel[r][:,0] (8KB) is gathered on host.
"""

import numpy as np

NE, NRR, D, C, B = 100000, 500, 512, 32, 2048
NCORES = 8
NQ = B // 128              # 16 batch tiles of 128
TPC = NQ // NCORES         # 2 batch tiles per core
JB = 126                   # window stride; window s covers x[126s-1 .. 126s+126]

_CACHE = {}


def _build_nc(n_nl):
    from contextlib import ExitStack

    import concourse.bass as bass
    import concourse.tile as tile
    from concourse import bacc, mybir
    from concourse.masks import make_identity

    f32 = mybir.dt.float32
    bf16 = mybir.dt.bfloat16
    i32 = mybir.dt.int32
    Alu = mybir.AluOpType

    nc = bacc.Bacc("TRN2", target_bir_lowering=False, debug=False,
                   num_devices=NCORES)

    ent = nc.dram_tensor("ent", [NE, D], bf16, kind="ExternalInput")
    hI = nc.dram_tensor("hI", [128, TPC], i32, kind="ExternalInput")
    tI = nc.dram_tensor("tI", [128, TPC], i32, kind="ExternalInput")
    relv = nc.dram_tensor("relv", [128, TPC], bf16, kind="ExternalInput")
    wfold = nc.dram_tensor("wfold", [128, 4 * D], bf16, kind="ExternalInput")
    stubf = nc.dram_tensor("stubf", [11, D], bf16, kind="ExternalInput")
    band = [nc.dram_tensor(f"band{i}", [128, 4 * JB], bf16,
                           kind="ExternalInput") for i in range(n_nl)]
    bstub = [nc.dram_tensor(f"bstub{i}", [10, 8], bf16,
                            kind="ExternalInput") for i in range(n_nl)]
    pwT = [nc.dram_tensor(f"pwT{i}", [JB, 4 * D], bf16,
                          kind="ExternalInput") for i in range(n_nl)]
    stub5 = [nc.dram_tensor(f"stub5_{i}", [8, D], bf16,
                            kind="ExternalInput") for i in range(n_nl)]
    cbc = [nc.dram_tensor(f"cbc{i}", [128, 1], f32,
                          kind="ExternalInput") for i in range(n_nl)]
    out = nc.dram_tensor("out", [128, TPC], f32, kind="ExternalOutput")

    with tile.TileContext(nc) as tc, ExitStack() as ctx:
        const = ctx.enter_context(tc.tile_pool(name="const", bufs=1))
        gpad_p = ctx.enter_context(tc.tile_pool(name="gpad", bufs=2))
        v_p = ctx.enter_context(tc.tile_pool(name="vt", bufs=2))
        gw_p = ctx.enter_context(tc.tile_pool(name="gw", bufs=2))
        gws_p = ctx.enter_context(tc.tile_pool(name="gws", bufs=2))
        ym_p = ctx.enter_context(tc.tile_pool(name="ym", bufs=2))
        ys_p = ctx.enter_context(tc.tile_pool(name="ys", bufs=2))
        sc_p = ctx.enter_context(tc.tile_pool(name="scr", bufs=2))
        tp_p = ctx.enter_context(tc.tile_pool(name="tp", bufs=2, space="PSUM"))
        y5_p = ctx.enter_context(tc.tile_pool(name="y5", bufs=2, space="PSUM"))
        yps_p = ctx.enter_context(tc.tile_pool(name="yps", bufs=2,
                                               space="PSUM"))
        z_p = ctx.enter_context(tc.tile_pool(name="zp", bufs=2, space="PSUM"))

        # index tables first: the gathers depend only on these
        hI_sb = const.tile([128, TPC], i32)
        nc.sync.dma_start(hI_sb[:], hI[:])
        tI_sb = const.tile([128, TPC], i32)
        nc.sync.dma_start(tI_sb[:], tI[:])

        # issue all gathers up front (gpsimd queue)
        gpads, vts = [], []
        for btl in range(TPC):
            gpad = gpad_p.tile([128, 515], bf16, name="gpad")
            nc.vector.memset(gpad[:, 0:1], 0.0)
            nc.vector.memset(gpad[:, 514:515], 1.0)
            nc.gpsimd.indirect_dma_start(
                out=gpad[:, 1:513], out_offset=None, in_=ent[:],
                in_offset=bass.IndirectOffsetOnAxis(
                    ap=hI_sb[:, btl:btl + 1], axis=0))
            nc.sync.dma_start(gpad[:, 513:514], relv[:, btl:btl + 1])
            gpads.append(gpad)
        for btl in range(TPC):
            vt = v_p.tile([128, D], f32, name="vt")
            nc.gpsimd.indirect_dma_start(
                out=vt[:], out_offset=None, in_=ent[:],
                in_offset=bass.IndirectOffsetOnAxis(
                    ap=tI_sb[:, btl:btl + 1], axis=0))
            vts.append(vt)

        ident = const.tile([128, 128], bf16)
        make_identity(nc, ident[:])
        wf_sb = const.tile([128, 4 * D], bf16)
        nc.sync.dma_start(wf_sb[:], wfold[:])
        stf_sb = const.tile([11, D], bf16)
        nc.sync.dma_start(stf_sb[:], stubf[:])
        band_sb, bstub_sb, pwT_sb, st5_sb, cb_sb = [], [], [], [], []
        for i in range(n_nl):
            b_ = const.tile([128, 4 * JB], bf16)
            nc.sync.dma_start(b_[:], band[i][:])
            band_sb.append(b_)
            bs = const.tile([10, 8], bf16)
            nc.sync.dma_start(bs[:], bstub[i][:])
            bstub_sb.append(bs)
            cbt = const.tile([128, 1], f32)
            nc.sync.dma_start(cbt[:], cbc[i][:])
            cb_sb.append(cbt)
            p_ = const.tile([JB, 4 * D], bf16)
            nc.sync.dma_start(p_[:], pwT[i][:])
            pwT_sb.append(p_)
            s5 = const.tile([8, D], bf16)
            nc.sync.dma_start(s5[:], stub5[i][:])
            st5_sb.append(s5)
        out_sb = const.tile([128, TPC], f32)

        for btl in range(TPC):
            gpad = gpads[btl]
            # transposes: 4 staggered 128-wide windows + 11-row stub
            tp = tp_p.tile([128, 640], bf16)
            for s in range(4):
                nc.tensor.transpose(tp[:, s * 128:(s + 1) * 128],
                                    gpad[:, JB * s:JB * s + 128], ident[:])
            nc.tensor.transpose(tp[0:11, 512:640], gpad[:, 504:515], ident[:])
            gw = gw_p.tile([128, 512], bf16)
            nc.vector.tensor_copy(gw[:], tp[:, 0:512])
            gws = gws_p.tile([11, 128], bf16)
            nc.vector.tensor_copy(gws[:], tp[0:11, 512:640])

            # nonlinear channels: banded conv + relu
            yms, ysts = [], []
            for i in range(n_nl):
                y5p = y5_p.tile([JB, 512], mybir.dt.float32, name="y5p")
                for s in range(4):
                    nc.tensor.matmul(y5p[:, s * 128:(s + 1) * 128],
                                     band_sb[i][:, s * JB:(s + 1) * JB],
                                     gw[:, s * 128:(s + 1) * 128],
                                     start=True, stop=True)
                yps = yps_p.tile([8, 128], mybir.dt.float32, name="yps")
                nc.tensor.matmul(yps[:], bstub_sb[i][:], gws[0:10, :],
                                 start=True, stop=True)
                ym = ym_p.tile([JB, 512], bf16, name="ym")
                nc.scalar.activation(ym[:], y5p[:],
                                     mybir.ActivationFunctionType.Relu,
                                     bias=cb_sb[i][0:JB, 0:1])
                yst = ys_p.tile([8, 128], bf16, name="yst")
                nc.scalar.activation(yst[:], yps[:],
                                     mybir.ActivationFunctionType.Relu,
                                     bias=cb_sb[i][0:8, 0:1])
                yms.append(ym)
                ysts.append(yst)

            # z accumulation: fold + fold-stub + per-NL proj + NL-stub
            z = z_p.tile([128, D], mybir.dt.float32, name="zt")
            for s in range(4):
                nc.tensor.matmul(z[:], gw[:, s * 128:(s + 1) * 128],
                                 wf_sb[:, s * D:(s + 1) * D],
                                 start=(s == 0), stop=False)
            nc.tensor.matmul(z[:], gws[:], stf_sb[:], start=False,
                             stop=(n_nl == 0))
            for i in range(n_nl):
                for s in range(4):
                    nc.tensor.matmul(z[:], yms[i][:, s * 128:(s + 1) * 128],
                                     pwT_sb[i][:, s * D:(s + 1) * D],
                                     start=False, stop=False)
                nc.tensor.matmul(z[:], ysts[i][:], st5_sb[i][:], start=False,
                                 stop=(i == n_nl - 1))

            scr = sc_p.tile([128, D], bf16)
            nc.vector.scalar_tensor_tensor(
                out=scr[:], in0=z[:], scalar=1.0, in1=vts[btl][:],
                op0=Alu.mult, op1=Alu.mult,
                accum_out=out_sb[:, btl:btl + 1])

        nc.sync.dma_start(out[:], out_sb[:])
    nc.finalize()
    return nc


def _host_prep(inputs):
    """Per-core input dicts + the exact relu fold, all from full inputs."""
    import ml_dtypes

    bf = ml_dtypes.bfloat16
    ent = np.asarray(inputs["ent"], dtype=np.float32)
    rel = np.asarray(inputs["rel"], dtype=np.float64)
    w = np.asarray(inputs["conv_w"], dtype=np.float64)     # [32, 1, 3]
    cb = np.asarray(inputs["conv_b"], dtype=np.float64)    # [32]
    pw = np.asarray(inputs["proj_w"], dtype=np.float64)    # [512, 16384]
    pb = np.asarray(inputs["proj_b"], dtype=np.float64)    # [512]
    h = np.asarray(inputs["h"]).astype(np.int32)
    r = np.asarray(inputs["r"]).astype(np.int32)
    t = np.asarray(inputs["t"]).astype(np.int32)

    ent_bf = np.ascontiguousarray(ent.astype(bf))

    # channel classification: relu provably identity / provably zero
    Me = float(np.abs(ent).max())
    Mr = float(np.abs(rel[:, 0]).max())
    aw = np.abs(w[:, 0, :])
    b_main = aw.sum(1) * Me
    b_last = (aw[:, 0] + aw[:, 1]) * Me + aw[:, 2] * Mr
    bound = np.maximum(b_main, b_last)
    lin = np.where(cb >= bound)[0]
    nl = np.where((cb < bound) & (cb > -bound))[0]
    n_nl = len(nl)

    # exact fold of linear channels: F [513, 512] over x, const [512]
    F = np.zeros((513, D))
    const = pb.copy()
    jg = np.arange(D)
    for c in lin:
        for k in range(3):
            i = jg + k - 1
            m = (i >= 0) & (i <= 512)
            F[i[m], :] += w[c, 0, k] * pw[:, c * D + jg[m]].T
        const += cb[c] * pw[:, c * D:(c + 1) * D].sum(1)

    # window scheme: window s partition p holds x[126s + p - 1]
    wfold = np.zeros((128, 4, D))
    for s in range(4):
        for p in range(JB):
            xi = JB * s + p - 1
            if 0 <= xi <= 502:
                wfold[p, s, :] = F[xi]
    stubf = np.zeros((11, D))
    stubf[0:10] = F[503:513]          # x[503..511] + rel row F[512]
    stubf[10] = const                 # ones row
    wfold = np.ascontiguousarray(wfold.reshape(128, 4 * D)).astype(bf)
    stubf = stubf.astype(bf)

    bands, bstubs, pwTs, stub5s, cbcs = [], [], [], [], []
    jl = np.arange(JB)
    jl8 = np.arange(8)
    for c in nl:
        bd = np.zeros((128, 4, JB))
        pT = np.zeros((JB, 4, D))
        for s in range(4):
            for k in range(3):
                bd[jl + k, s, jl] = w[c, 0, k]
            pT[:, s, :] = pw[:, c * D + JB * s: c * D + JB * (s + 1)].T
        bs = np.zeros((10, 8))
        for k in range(3):
            bs[jl8 + k, jl8] = w[c, 0, k]
        s5 = pw[:, c * D + 504: c * D + 512].T
        bands.append(np.ascontiguousarray(bd.reshape(128, 4 * JB)).astype(bf))
        pwTs.append(np.ascontiguousarray(pT.reshape(JB, 4 * D)).astype(bf))
        bstubs.append(bs.astype(bf))
        stub5s.append(np.ascontiguousarray(s5).astype(bf))
        cbcs.append(np.full((128, 1), cb[c], np.float32))

    hI = np.ascontiguousarray(h.reshape(NQ, 128).T)
    tI = np.ascontiguousarray(t.reshape(NQ, 128).T)
    relv = np.ascontiguousarray(
        rel[r, 0].astype(np.float32).reshape(NQ, 128).T.astype(bf))

    in_maps = []
    for m in range(NCORES):
        sl = slice(m * TPC, (m + 1) * TPC)
        d = {
            "ent": ent_bf,
            "hI": np.ascontiguousarray(hI[:, sl]),
            "tI": np.ascontiguousarray(tI[:, sl]),
            "relv": np.ascontiguousarray(relv[:, sl]),
            "wfold": wfold, "stubf": stubf,
        }
        for i in range(n_nl):
            d[f"band{i}"] = bands[i]
            d[f"bstub{i}"] = bstubs[i]
            d[f"pwT{i}"] = pwTs[i]
            d[f"stub5_{i}"] = stub5s[i]
            d[f"cbc{i}"] = cbcs[i]
        in_maps.append(d)
    return in_maps, n_nl


def _run(inputs, trace=False, tmpdir=None):
    from concourse.bass_utils import run_bass_kernel_spmd

    in_maps, n_nl = _host_prep(inputs)
    key = ("nc", n_nl)
    if key not in _CACHE:
        _CACHE[key] = _build_nc(n_nl)
    nc = _CACHE[key]
    res = run_bass_kernel_spmd(nc, in_maps, core_ids=list(range(NCORES)),
                               trace=trace, tmpdir=tmpdir)
    total = np.zeros((128, NQ), np.float32)
    for m, mres in enumerate(res.results):
        total[:, m * TPC:(m + 1) * TPC] = mres["out"]
    return total.T.reshape(B), res


def kernel(**inputs):
    out, _ = _run(inputs, trace=False)
    return out


# revision 4
# speedup vs baseline: 3.7117x; 1.1010x over previous
"""ConvTransE forward on 8 Trainium2 NeuronCores (Bass/Tile) — v9 "relu fold".

Math: the reference returns out[b] = z[b] . ent[t[b]] with
z = relu(conv(x) + cb) @ proj_w.T + pb, x = [ent[h], rel[r][:,0-only]].
Because |ent| <= 0.0077 while conv_b ~ U(-0.58, 0.58), for most channels
relu is provably the identity (cb >= max possible |conv|) or provably
zero (cb <= -max).  Identity channels fold EXACTLY into one linear map
W_fold [513+ones, 512] built on host from the weights; zero channels
drop; only the few genuinely nonlinear channels keep the
conv->relu->proj path on device.  With the staged weights that is 23
linear / 8 zero / 1 nonlinear channel.

Sharding: data-parallel over batch.  Core m owns batch tiles 2m, 2m+1
(256 rows).  Per tile: indirect-gather ent[h] rows (bf16), PE-transpose
126-stride windows, z = xT.T @ W_fold (+ stub for x[503..512], rel,
ones/const rows), plus per-NL-channel banded conv (one matmul — the
band is segment-independent) + relu + projection, then
out[b] = z[b] . ent[t[b]] via fused multiply+row-sum.  No cross-core
reduction; host concatenates the per-core [128, 2] outputs.
rel[r][:,0] (8KB) is gathered on host.

v9 schedule: the first h-gather's index-load + descriptor-gen + 28
ns/row DMA keeps data away from the PE until ~14.5us while the Tile
preamble ends ~7.2us; that window is filled with dummy identity
transposes so the PE p-state ramps to full clock before real work.
Weights ride in two packed bf16 DMAs on two engine queues; conv bias
is baked as a memset; ACT table preloaded with a dummy activation.
"""

import numpy as np

NE, NRR, D, C, B = 100000, 500, 512, 32, 2048
NCORES = 8
NQ = B // 128              # 16 batch tiles of 128
TPC = NQ // NCORES         # 2 batch tiles per core
JB = 126                   # window stride; window s covers x[126s-1 .. 126s+126]
NWARM = 28                 # PE p-state warmup transposes

_CACHE = {}


def _build_nc(n_nl, cbvals):
    from contextlib import ExitStack

    import concourse.bass as bass
    import concourse.tile as tile
    from concourse import bacc, mybir
    from concourse.masks import make_identity

    f32 = mybir.dt.float32
    bf16 = mybir.dt.bfloat16
    i32 = mybir.dt.int32
    Alu = mybir.AluOpType

    nc = bacc.Bacc("TRN2", target_bir_lowering=False, debug=False,
                   num_devices=NCORES)

    # w1: [wfold 4*D | band n*126]  (128 partitions)
    # w2: [pwT n*4*D (126p) | stubf D (11p) | stub5 n*D (8p) | bstub n*8 (10p)
    #      | relv TPC (128p)]
    W1 = 4 * D + n_nl * JB
    W2 = n_nl * 4 * D + D + n_nl * D + n_nl * 8 + TPC
    ent = nc.dram_tensor("ent", [NE, D], bf16, kind="ExternalInput")
    idx = nc.dram_tensor("idx", [128, 2 * TPC], i32, kind="ExternalInput")
    w1 = nc.dram_tensor("w1", [128, W1], bf16, kind="ExternalInput")
    w2 = nc.dram_tensor("w2", [128, W2], bf16, kind="ExternalInput")
    out = nc.dram_tensor("out", [128, TPC], f32, kind="ExternalOutput")

    with tile.TileContext(nc) as tc, ExitStack() as ctx:
        const = ctx.enter_context(tc.tile_pool(name="const", bufs=1))
        gpad_p = ctx.enter_context(tc.tile_pool(name="gpad", bufs=2))
        v_p = ctx.enter_context(tc.tile_pool(name="vt", bufs=2))
        gw_p = ctx.enter_context(tc.tile_pool(name="gw", bufs=2))
        ym_p = ctx.enter_context(tc.tile_pool(name="ym", bufs=2))
        ys_p = ctx.enter_context(tc.tile_pool(name="ys", bufs=2))
        sc_p = ctx.enter_context(tc.tile_pool(name="scr", bufs=2))
        tp_p = ctx.enter_context(tc.tile_pool(name="tp", bufs=2, space="PSUM"))
        y5_p = ctx.enter_context(tc.tile_pool(name="y5", bufs=1, space="PSUM"))
        yps_p = ctx.enter_context(tc.tile_pool(name="yps", bufs=1,
                                               space="PSUM"))
        z_p = ctx.enter_context(tc.tile_pool(name="zp", bufs=2, space="PSUM"))

        # index table first: the gathers depend only on this
        idx_sb = const.tile([128, 2 * TPC], i32)
        nc.sync.dma_start(idx_sb[:], idx[:])
        # packed weights: two queues so the transfers overlap
        w1_sb = const.tile([128, W1], bf16)
        nc.sync.dma_start(w1_sb[:], w1[:])
        w2_sb = const.tile([128, W2], bf16)
        nc.scalar.dma_start(w2_sb[:], w2[:])

        wf_sb = w1_sb[:, 0:4 * D]
        band_sb = [w1_sb[:, 4 * D + i * JB:4 * D + (i + 1) * JB]
                   for i in range(n_nl)]
        pwT_sb = [w2_sb[0:JB, i * 4 * D:(i + 1) * 4 * D] for i in range(n_nl)]
        o = n_nl * 4 * D
        stf_sb = w2_sb[0:11, o:o + D]
        st5_sb = [w2_sb[0:8, o + D + i * D:o + D + (i + 1) * D]
                  for i in range(n_nl)]
        o2 = o + D + n_nl * D
        bstub_sb = [w2_sb[0:10, o2 + i * 8:o2 + (i + 1) * 8]
                    for i in range(n_nl)]
        relv_sb = w2_sb[:, o2 + n_nl * 8:o2 + n_nl * 8 + TPC]

        # issue all gathers up front (gpsimd queue)
        gpads, vts = [], []
        for btl in range(TPC):
            gpad = gpad_p.tile([128, 515], bf16, name="gpad")
            nc.vector.memset(gpad[:, 0:1], 0.0)
            nc.vector.memset(gpad[:, 514:515], 1.0)
            nc.gpsimd.indirect_dma_start(
                out=gpad[:, 1:513], out_offset=None, in_=ent[:],
                in_offset=bass.IndirectOffsetOnAxis(
                    ap=idx_sb[:, btl:btl + 1], axis=0))
            gpads.append(gpad)
        for btl in range(TPC):
            vt = v_p.tile([128, D], f32, name="vt")
            nc.gpsimd.indirect_dma_start(
                out=vt[:], out_offset=None, in_=ent[:],
                in_offset=bass.IndirectOffsetOnAxis(
                    ap=idx_sb[:, TPC + btl:TPC + btl + 1], axis=0))
            vts.append(vt)

        ident = const.tile([128, 128], bf16)
        make_identity(nc, ident[:])
        cb_sb = []
        for i in range(n_nl):
            cbt = const.tile([128, 1], f32)
            nc.vector.memset(cbt[:], float(cbvals[i]))
            cb_sb.append(cbt)
        # preload the activation table so the first real relu is cheap
        actw = const.tile([1, 1], f32)
        nc.scalar.activation(actw[:], ident[0:1, 0:1],
                             mybir.ActivationFunctionType.Relu)
        out_sb = const.tile([128, TPC], f32)

        # PE p-state warmup: dep-free transposes while the gather lands
        warm = tp_p.tile([128, 640], bf16)
        for i in range(NWARM):
            nc.tensor.transpose(warm[:, (i % 4) * 128:(i % 4 + 1) * 128],
                                ident[:], ident[:])

        for btl in range(TPC):
            gpad = gpads[btl]
            # rel value for x[512] rides in w2; ones via memset above
            nc.vector.tensor_copy(gpad[:, 513:514],
                                  relv_sb[:, btl:btl + 1])
            # transposes: 4 staggered 128-wide windows + 11-row stub
            tp = tp_p.tile([128, 640], bf16)
            for s in range(4):
                nc.tensor.transpose(tp[:, s * 128:(s + 1) * 128],
                                    gpad[:, JB * s:JB * s + 128], ident[:])
            nc.tensor.transpose(tp[0:11, 512:640], gpad[:, 504:515], ident[:])
            gw = gw_p.tile([128, 640], bf16)
            nc.vector.tensor_copy(gw[:], tp[:])

            # nonlinear channels: banded conv (one matmul) + relu
            yms, ysts = [], []
            for i in range(n_nl):
                y5p = y5_p.tile([JB, 512], mybir.dt.float32, name="y5p")
                nc.tensor.matmul(y5p[:], band_sb[i], gw[:, 0:512],
                                 start=True, stop=True)
                yps = yps_p.tile([8, 128], mybir.dt.float32, name="yps")
                nc.tensor.matmul(yps[:], bstub_sb[i], gw[0:10, 512:640],
                                 start=True, stop=True)
                ym = ym_p.tile([JB, 512], bf16, name="ym")
                nc.scalar.activation(ym[:], y5p[:],
                                     mybir.ActivationFunctionType.Relu,
                                     bias=cb_sb[i][0:JB, 0:1])
                yst = ys_p.tile([8, 128], bf16, name="yst")
                nc.scalar.activation(yst[:], yps[:],
                                     mybir.ActivationFunctionType.Relu,
                                     bias=cb_sb[i][0:8, 0:1])
                yms.append(ym)
                ysts.append(yst)

            # z accumulation: fold + fold-stub + per-NL proj + NL-stub
            z = z_p.tile([128, D], mybir.dt.float32, name="zt")
            for s in range(4):
                nc.tensor.matmul(z[:], gw[:, s * 128:(s + 1) * 128],
                                 wf_sb[:, s * D:(s + 1) * D],
                                 start=(s == 0), stop=False)
            nc.tensor.matmul(z[:], gw[0:11, 512:640], stf_sb, start=False,
                             stop=(n_nl == 0))
            for i in range(n_nl):
                for s in range(4):
                    nc.tensor.matmul(z[:], yms[i][:, s * 128:(s + 1) * 128],
                                     pwT_sb[i][:, s * D:(s + 1) * D],
                                     start=False, stop=False)
                nc.tensor.matmul(z[:], ysts[i][:], st5_sb[i], start=False,
                                 stop=(i == n_nl - 1))

            scr = sc_p.tile([128, D], bf16)
            nc.vector.scalar_tensor_tensor(
                out=scr[:], in0=z[:], scalar=1.0, in1=vts[btl][:],
                op0=Alu.mult, op1=Alu.mult,
                accum_out=out_sb[:, btl:btl + 1])
            nc.sync.dma_start(out[:, btl:btl + 1], out_sb[:, btl:btl + 1])
    nc.finalize()
    return nc


def _host_prep(inputs):
    """Per-core input dicts + the exact relu fold, all from full inputs."""
    import ml_dtypes

    bf = ml_dtypes.bfloat16
    ent = np.asarray(inputs["ent"], dtype=np.float32)
    rel = np.asarray(inputs["rel"], dtype=np.float64)
    w = np.asarray(inputs["conv_w"], dtype=np.float64)     # [32, 1, 3]
    cb = np.asarray(inputs["conv_b"], dtype=np.float64)    # [32]
    pw = np.asarray(inputs["proj_w"], dtype=np.float64)    # [512, 16384]
    pb = np.asarray(inputs["proj_b"], dtype=np.float64)    # [512]
    h = np.asarray(inputs["h"]).astype(np.int32)
    r = np.asarray(inputs["r"]).astype(np.int32)
    t = np.asarray(inputs["t"]).astype(np.int32)

    ent_bf = np.ascontiguousarray(ent.astype(bf))

    # channel classification: relu provably identity / provably zero
    Me = float(np.abs(ent).max())
    Mr = float(np.abs(rel[:, 0]).max())
    aw = np.abs(w[:, 0, :])
    b_main = aw.sum(1) * Me
    b_last = (aw[:, 0] + aw[:, 1]) * Me + aw[:, 2] * Mr
    bound = np.maximum(b_main, b_last)
    lin = np.where(cb >= bound)[0]
    nl = np.where((cb < bound) & (cb > -bound))[0]
    n_nl = len(nl)

    # exact fold of linear channels: F [513, 512] over x, const [512]
    F = np.zeros((513, D))
    const = pb.copy()
    jg = np.arange(D)
    for c in lin:
        for k in range(3):
            i = jg + k - 1
            m = (i >= 0) & (i <= 512)
            F[i[m], :] += w[c, 0, k] * pw[:, c * D + jg[m]].T
        const += cb[c] * pw[:, c * D:(c + 1) * D].sum(1)

    # window scheme: window s partition p holds x[126s + p - 1]
    wfold = np.zeros((128, 4, D))
    for s in range(4):
        for p in range(JB):
            xi = JB * s + p - 1
            if 0 <= xi <= 502:
                wfold[p, s, :] = F[xi]
    stubf = np.zeros((11, D))
    stubf[0:10] = F[503:513]          # x[503..511] + rel row F[512]
    stubf[10] = const                 # ones row

    jl = np.arange(JB)
    jl8 = np.arange(8)
    bands, bstubs, pwTs, stub5s = [], [], [], []
    for c in nl:
        bd = np.zeros((128, JB))      # segment-independent band
        bs = np.zeros((10, 8))
        for k in range(3):
            bd[jl + k, jl] = w[c, 0, k]
            bs[jl8 + k, jl8] = w[c, 0, k]
        pT = np.zeros((JB, 4, D))
        for s in range(4):
            pT[:, s, :] = pw[:, c * D + JB * s: c * D + JB * (s + 1)].T
        bands.append(bd)
        bstubs.append(bs)
        pwTs.append(pT.reshape(JB, 4 * D))
        stub5s.append(pw[:, c * D + 504: c * D + 512].T)

    hI = np.ascontiguousarray(h.reshape(NQ, 128).T)
    tI = np.ascontiguousarray(t.reshape(NQ, 128).T)
    relv = rel[r, 0].astype(np.float32).reshape(NQ, 128).T

    # pack weights: w1 [128, 4D + n*126], w2 [128, n*4D + D + n*D + n*8 + TPC]
    W1 = 4 * D + n_nl * JB
    W2 = n_nl * 4 * D + D + n_nl * D + n_nl * 8 + TPC
    w1p = np.zeros((128, W1))
    w1p[:, 0:4 * D] = wfold.reshape(128, 4 * D)
    for i in range(n_nl):
        w1p[:, 4 * D + i * JB:4 * D + (i + 1) * JB] = bands[i]
    w2_common = np.zeros((128, W2))
    for i in range(n_nl):
        w2_common[0:JB, i * 4 * D:(i + 1) * 4 * D] = pwTs[i]
    o = n_nl * 4 * D
    w2_common[0:11, o:o + D] = stubf
    for i in range(n_nl):
        w2_common[0:8, o + D + i * D:o + D + (i + 1) * D] = stub5s[i]
    o2 = o + D + n_nl * D
    for i in range(n_nl):
        w2_common[0:10, o2 + i * 8:o2 + (i + 1) * 8] = bstubs[i]
    w1p = w1p.astype(bf)

    in_maps = []
    for m in range(NCORES):
        sl = slice(m * TPC, (m + 1) * TPC)
        idxm = np.concatenate([hI[:, sl], tI[:, sl]], axis=1)
        w2p = w2_common.copy()
        w2p[:, o2 + n_nl * 8:o2 + n_nl * 8 + TPC] = relv[:, sl]
        in_maps.append({
            "ent": ent_bf,
            "idx": np.ascontiguousarray(idxm),
            "w1": w1p,
            "w2": np.ascontiguousarray(w2p.astype(bf)),
        })
    return in_maps, n_nl, tuple(float(cb[c]) for c in nl)


def _run(inputs, trace=False, tmpdir=None):
    from concourse.bass_utils import run_bass_kernel_spmd

    in_maps, n_nl, cbvals = _host_prep(inputs)
    key = ("nc", n_nl, cbvals)
    if key not in _CACHE:
        _CACHE[key] = _build_nc(n_nl, cbvals)
    nc = _CACHE[key]
    res = run_bass_kernel_spmd(nc, in_maps, core_ids=list(range(NCORES)),
                               trace=trace, tmpdir=tmpdir)
    total = np.zeros((128, NQ), np.float32)
    for m, mres in enumerate(res.results):
        total[:, m * TPC:(m + 1) * TPC] = mres["out"]
    return total.T.reshape(B), res


def kernel(**inputs):
    out, _ = _run(inputs, trace=False)
    return out


# revision 13
# speedup vs baseline: 3.9357x; 1.0603x over previous
"""ConvTransE forward on 8 Trainium2 NeuronCores (Bass/Tile) — v10 "relu fold".

Math: the reference returns out[b] = z[b] . ent[t[b]] with
z = relu(conv(x) + cb) @ proj_w.T + pb, x = [ent[h], rel[r][:,0-only]].
Because |ent| <= 0.0077 while conv_b ~ U(-0.58, 0.58), for most channels
relu is provably the identity (cb >= max possible |conv|) or provably
zero (cb <= -max).  Identity channels fold EXACTLY into one linear map
W_fold [513+ones, 512] built on host from the weights; zero channels
drop; only the few genuinely nonlinear channels keep the
conv->relu->proj path on device.  With the staged weights that is 23
linear / 8 zero / 1 nonlinear channel.

Sharding: data-parallel over batch.  Core m owns batch tiles 2m, 2m+1
(256 rows).  Per tile: indirect-gather ent[h] rows (bf16), PE-transpose
126-stride windows, z = xT.T @ W_fold (+ stub for x[503..512], rel,
ones/const rows), plus per-NL-channel banded conv (one matmul — the
band is segment-independent) + relu + projection, then
out[b] = z[b] . ent[t[b]] via fused multiply+row-sum.  No cross-core
reduction; host concatenates the per-core [TPC, 128] outputs.
rel[r][:,0] (8KB) is gathered on host.

v10 latency schedule (everything [128, few-cols] is poison — DMA
splits per-partition lines into 4..32-byte packets that trickle for
microseconds):
  - the output is PE-transposed to [TPC, 128] f32 before the store.
  - stubf/relv ride in w1 (early queue), pwT/stub5/bstub in w2.
  - dummy 512-col matmuls ramp the PE p-state (clock doubles after
    ~3.4us of activity) while the first gather is in flight.
"""

import numpy as np

NE, NRR, D, C, B = 100000, 500, 512, 32, 2048
NCORES = 8
NQ = B // 128              # 16 batch tiles of 128
TPC = NQ // NCORES         # 2 batch tiles per core
JB = 126                   # window stride; window s covers x[126s-1 .. 126s+126]
NWARM = 34                 # PE p-state warmup transposes

_CACHE = {}


def _build_nc(n_nl, cbvals):
    from contextlib import ExitStack

    import concourse.bass as bass
    import concourse.tile as tile
    from concourse import bacc, mybir
    from concourse.masks import make_identity

    f32 = mybir.dt.float32
    bf16 = mybir.dt.bfloat16
    i32 = mybir.dt.int32
    Alu = mybir.AluOpType

    nc = bacc.Bacc("TRN2", target_bir_lowering=False, debug=False,
                   num_devices=NCORES)

    # w1: [wfold 4*D | band n*126 | stubf D (11p) | relv TPC (128p)]
    # w2: [pwT n*4*D (126p) | stub5 n*D (8p) | bstub n*8 (10p)]
    W1 = 4 * D + n_nl * JB + D + TPC
    W2 = n_nl * 4 * D + n_nl * D + n_nl * 8
    ent = nc.dram_tensor("ent", [NE, D], bf16, kind="ExternalInput")
    idx = nc.dram_tensor("idx", [128, 2 * TPC], i32, kind="ExternalInput")
    w1 = nc.dram_tensor("w1", [128, W1], bf16, kind="ExternalInput")
    w2 = nc.dram_tensor("w2", [128, max(W2, 1)], bf16, kind="ExternalInput")
    out = nc.dram_tensor("out", [TPC, 128], f32, kind="ExternalOutput")

    with tile.TileContext(nc) as tc, ExitStack() as ctx:
        const = ctx.enter_context(tc.tile_pool(name="const", bufs=1))
        gpad_p = ctx.enter_context(tc.tile_pool(name="gpad", bufs=2))
        v_p = ctx.enter_context(tc.tile_pool(name="vt", bufs=2))
        gw_p = ctx.enter_context(tc.tile_pool(name="gw", bufs=2))
        ym_p = ctx.enter_context(tc.tile_pool(name="ym", bufs=2))
        ys_p = ctx.enter_context(tc.tile_pool(name="ys", bufs=2))
        sc_p = ctx.enter_context(tc.tile_pool(name="scr", bufs=2))
        tp_p = ctx.enter_context(tc.tile_pool(name="tp", bufs=2, space="PSUM"))
        ts_p = ctx.enter_context(tc.tile_pool(name="ts", bufs=1, space="PSUM"))
        y5_p = ctx.enter_context(tc.tile_pool(name="y5", bufs=1, space="PSUM"))
        yps_p = ctx.enter_context(tc.tile_pool(name="yps", bufs=1,
                                               space="PSUM"))
        z_p = ctx.enter_context(tc.tile_pool(name="zp", bufs=2, space="PSUM"))

        # index row first: the gathers depend only on this (single
        # partition -> single DMA packet, no per-line trickle)
        idx_sb = const.tile([128, 2 * TPC], i32)
        nc.sync.dma_start(idx_sb[:], idx[:])
        # packed weights: two queues so the transfers overlap
        w1_sb = const.tile([128, W1], bf16)
        nc.sync.dma_start(w1_sb[:], w1[:])
        w2_sb = const.tile([128, max(W2, 1)], bf16)
        nc.scalar.dma_start(w2_sb[:], w2[:])

        wf_sb = w1_sb[:, 0:4 * D]
        band_sb = [w1_sb[:, 4 * D + i * JB:4 * D + (i + 1) * JB]
                   for i in range(n_nl)]
        o1 = 4 * D + n_nl * JB
        stf_sb = w1_sb[0:11, o1:o1 + D]
        relv_sb = w1_sb[:, o1 + D:o1 + D + TPC]
        pwT_sb = [w2_sb[0:JB, i * 4 * D:(i + 1) * 4 * D] for i in range(n_nl)]
        o = n_nl * 4 * D
        st5_sb = [w2_sb[0:8, o + i * D:o + (i + 1) * D] for i in range(n_nl)]
        o2 = o + n_nl * D
        bstub_sb = [w2_sb[0:10, o2 + i * 8:o2 + (i + 1) * 8]
                    for i in range(n_nl)]

        # issue all gathers up front (gpsimd queue)
        gpads, vts = [], []
        for btl in range(TPC):
            gpad = gpad_p.tile([128, 515], bf16, name="gpad")
            nc.vector.memset(gpad[:, 0:1], 0.0)
            nc.vector.memset(gpad[:, 514:515], 1.0)
            nc.gpsimd.indirect_dma_start(
                out=gpad[:, 1:513], out_offset=None, in_=ent[:],
                in_offset=bass.IndirectOffsetOnAxis(
                    ap=idx_sb[:, btl:btl + 1], axis=0))
            gpads.append(gpad)
        for btl in range(TPC):
            vt = v_p.tile([128, D], f32, name="vt")
            nc.gpsimd.indirect_dma_start(
                out=vt[:], out_offset=None, in_=ent[:],
                in_offset=bass.IndirectOffsetOnAxis(
                    ap=idx_sb[:, TPC + btl:TPC + btl + 1], axis=0))
            vts.append(vt)

        ident = const.tile([128, 128], bf16)
        make_identity(nc, ident[:])
        cb_sb = []
        for i in range(n_nl):
            cbt = const.tile([128, 1], f32)
            nc.vector.memset(cbt[:], float(cbvals[i]))
            cb_sb.append(cbt)
        # preload the activation table so the first real relu is cheap
        actw = const.tile([1, 1], f32)
        nc.scalar.activation(actw[:], ident[0:1, 0:1],
                             mybir.ActivationFunctionType.Relu)
        out_sb = const.tile([128, TPC], f32)
        outT_sb = const.tile([TPC, 128], f32)

        # PE p-state warmup: dep-free transposes while the gather lands
        warmp = tp_p.tile([128, 512], bf16, tag="scr", bufs=1)
        for i in range(NWARM):
            nc.tensor.transpose(warmp[:, (i % 4) * 128:(i % 4 + 1) * 128],
                                ident[:], ident[:])

        for btl in range(TPC):
            gpad = gpads[btl]
            # rel value for x[512] rides in w1; ones via memset above
            nc.vector.tensor_copy(gpad[:, 513:514],
                                  relv_sb[:, btl:btl + 1])
            # transposes: 4 staggered 128-wide windows + 11-row stub
            tp = tp_p.tile([128, 512], bf16)
            for s in range(4):
                nc.tensor.transpose(tp[:, s * 128:(s + 1) * 128],
                                    gpad[:, JB * s:JB * s + 128], ident[:])
            tps = ts_p.tile([11, 128], bf16)
            nc.tensor.transpose(tps[:], gpad[:, 504:515], ident[:])
            gw = gw_p.tile([128, 640], bf16)
            nc.vector.tensor_copy(gw[:, 0:512], tp[:])
            nc.vector.tensor_copy(gw[0:11, 512:640], tps[:])

            # nonlinear channels: banded conv (one matmul) + relu
            yms, ysts = [], []
            for i in range(n_nl):
                y5p = y5_p.tile([JB, 512], mybir.dt.float32, name="y5p")
                nc.tensor.matmul(y5p[:], band_sb[i], gw[:, 0:512],
                                 start=True, stop=True)
                yps = yps_p.tile([8, 128], mybir.dt.float32, name="yps")
                nc.tensor.matmul(yps[:], bstub_sb[i], gw[0:10, 512:640],
                                 start=True, stop=True)
                ym = ym_p.tile([JB, 512], bf16, name="ym")
                nc.scalar.activation(ym[:], y5p[:],
                                     mybir.ActivationFunctionType.Relu,
                                     bias=cb_sb[i][0:JB, 0:1])
                yst = ys_p.tile([8, 128], bf16, name="yst")
                nc.scalar.activation(yst[:], yps[:],
                                     mybir.ActivationFunctionType.Relu,
                                     bias=cb_sb[i][0:8, 0:1])
                yms.append(ym)
                ysts.append(yst)

            # z accumulation: fold + fold-stub + per-NL proj + NL-stub
            z = z_p.tile([128, D], mybir.dt.float32, name="zt")
            for s in range(4):
                nc.tensor.matmul(z[:], gw[:, s * 128:(s + 1) * 128],
                                 wf_sb[:, s * D:(s + 1) * D],
                                 start=(s == 0), stop=False)
            nc.tensor.matmul(z[:], gw[0:11, 512:640], stf_sb, start=False,
                             stop=(n_nl == 0))
            for i in range(n_nl):
                for s in range(4):
                    nc.tensor.matmul(z[:], yms[i][:, s * 128:(s + 1) * 128],
                                     pwT_sb[i][:, s * D:(s + 1) * D],
                                     start=False, stop=False)
                nc.tensor.matmul(z[:], ysts[i][:], st5_sb[i], start=False,
                                 stop=(i == n_nl - 1))

            scr = sc_p.tile([128, D], bf16)
            nc.vector.scalar_tensor_tensor(
                out=scr[:], in0=z[:], scalar=1.0, in1=vts[btl][:],
                op0=Alu.mult, op1=Alu.mult,
                accum_out=out_sb[:, btl:btl + 1])

        # transpose the output so the store is [TPC, 128]: contiguous
        # 512B lines instead of 128 4-byte packets
        identf = const.tile([128, 128], mybir.dt.float32)
        nc.vector.tensor_copy(identf[:], ident[:])
        otp = tp_p.tile([TPC, 128], mybir.dt.float32, tag="scr", bufs=1)
        nc.tensor.transpose(otp[:], out_sb[:], identf[:])
        nc.vector.tensor_copy(outT_sb[:], otp[:])
        nc.sync.dma_start(out[:], outT_sb[:])
    nc.finalize()
    return nc


def _host_prep(inputs):
    """Per-core input dicts + the exact relu fold, all from full inputs."""
    import ml_dtypes

    bf = ml_dtypes.bfloat16
    ent = np.asarray(inputs["ent"], dtype=np.float32)
    rel = np.asarray(inputs["rel"], dtype=np.float64)
    w = np.asarray(inputs["conv_w"], dtype=np.float64)     # [32, 1, 3]
    cb = np.asarray(inputs["conv_b"], dtype=np.float64)    # [32]
    pw = np.asarray(inputs["proj_w"], dtype=np.float64)    # [512, 16384]
    pb = np.asarray(inputs["proj_b"], dtype=np.float64)    # [512]
    h = np.asarray(inputs["h"]).astype(np.int32)
    r = np.asarray(inputs["r"]).astype(np.int32)
    t = np.asarray(inputs["t"]).astype(np.int32)

    ent_bf = np.ascontiguousarray(ent.astype(bf))

    # channel classification: relu provably identity / provably zero
    Me = float(np.abs(ent).max())
    Mr = float(np.abs(rel[:, 0]).max())
    aw = np.abs(w[:, 0, :])
    b_main = aw.sum(1) * Me
    b_last = (aw[:, 0] + aw[:, 1]) * Me + aw[:, 2] * Mr
    bound = np.maximum(b_main, b_last)
    lin = np.where(cb >= bound)[0]
    nl = np.where((cb < bound) & (cb > -bound))[0]
    n_nl = len(nl)

    # exact fold of linear channels: F [513, 512] over x, const [512]
    F = np.zeros((513, D))
    const = pb.copy()
    jg = np.arange(D)
    for c in lin:
        for k in range(3):
            i = jg + k - 1
            m = (i >= 0) & (i <= 512)
            F[i[m], :] += w[c, 0, k] * pw[:, c * D + jg[m]].T
        const += cb[c] * pw[:, c * D:(c + 1) * D].sum(1)

    # window scheme: window s partition p holds x[126s + p - 1]
    wfold = np.zeros((128, 4, D))
    for s in range(4):
        for p in range(JB):
            xi = JB * s + p - 1
            if 0 <= xi <= 502:
                wfold[p, s, :] = F[xi]
    stubf = np.zeros((11, D))
    stubf[0:10] = F[503:513]          # x[503..511] + rel row F[512]
    stubf[10] = const                 # ones row

    jl = np.arange(JB)
    jl8 = np.arange(8)
    bands, bstubs, pwTs, stub5s = [], [], [], []
    for c in nl:
        bd = np.zeros((128, JB))      # segment-independent band
        bs = np.zeros((10, 8))
        for k in range(3):
            bd[jl + k, jl] = w[c, 0, k]
            bs[jl8 + k, jl8] = w[c, 0, k]
        pT = np.zeros((JB, 4, D))
        for s in range(4):
            pT[:, s, :] = pw[:, c * D + JB * s: c * D + JB * (s + 1)].T
        bands.append(bd)
        bstubs.append(bs)
        pwTs.append(pT.reshape(JB, 4 * D))
        stub5s.append(pw[:, c * D + 504: c * D + 512].T)

    hI = h.reshape(NQ, 128)
    tI = t.reshape(NQ, 128)
    relv = rel[r, 0].astype(np.float32).reshape(NQ, 128).T

    # pack weights
    W1 = 4 * D + n_nl * JB + D + TPC
    W2 = n_nl * 4 * D + n_nl * D + n_nl * 8
    w1_common = np.zeros((128, W1))
    w1_common[:, 0:4 * D] = wfold.reshape(128, 4 * D)
    for i in range(n_nl):
        w1_common[:, 4 * D + i * JB:4 * D + (i + 1) * JB] = bands[i]
    o1 = 4 * D + n_nl * JB
    w1_common[0:11, o1:o1 + D] = stubf
    w2p = np.zeros((128, max(W2, 1)))
    for i in range(n_nl):
        w2p[0:JB, i * 4 * D:(i + 1) * 4 * D] = pwTs[i]
    o = n_nl * 4 * D
    for i in range(n_nl):
        w2p[0:8, o + i * D:o + (i + 1) * D] = stub5s[i]
    o2 = o + n_nl * D
    for i in range(n_nl):
        w2p[0:10, o2 + i * 8:o2 + (i + 1) * 8] = bstubs[i]
    w2p = np.ascontiguousarray(w2p.astype(bf))

    in_maps = []
    for m in range(NCORES):
        sl = slice(m * TPC, (m + 1) * TPC)
        idxm = np.concatenate([hI[sl].T, tI[sl].T], axis=1)
        w1p = w1_common.copy()
        w1p[:, o1 + D:o1 + D + TPC] = relv[:, sl]
        in_maps.append({
            "ent": ent_bf,
            "idx": np.ascontiguousarray(idxm),
            "w1": np.ascontiguousarray(w1p.astype(bf)),
            "w2": w2p,
        })
    return in_maps, n_nl, tuple(float(cb[c]) for c in nl)


def _run(inputs, trace=False, tmpdir=None):
    from concourse.bass_utils import run_bass_kernel_spmd

    in_maps, n_nl, cbvals = _host_prep(inputs)
    key = ("nc", n_nl, cbvals)
    if key not in _CACHE:
        _CACHE[key] = _build_nc(n_nl, cbvals)
    nc = _CACHE[key]
    res = run_bass_kernel_spmd(nc, in_maps, core_ids=list(range(NCORES)),
                               trace=trace, tmpdir=tmpdir)
    total = np.zeros((NQ, 128), np.float32)
    for m, mres in enumerate(res.results):
        total[m * TPC:(m + 1) * TPC, :] = mres["out"]
    return total.reshape(B), res


def kernel(**inputs):
    out, _ = _run(inputs, trace=False)
    return out
